# revision 2
# baseline (speedup 1.0000x reference)
"""Trainium2 Bass kernel for the CIN block:
out[b,o,k] = sum_{h,m} W[o, h*M+m] * xl[b,h,k] * x0[b,m,k] + bias[o]

Strategy (data-parallel over batch across 8 cores, 32 batches/core,
processed in 8 groups of 4 batches; GEMM operands bf16, fp32 PSUM):
  - fmap chunk p (rows c=128p..128p+128, c=(h,m)) is built in [C, K]
    layout: a contraction-2 matmul broadcasts the two xl rows of the
    chunk into PSUM.  Chunks are emitted as 2-chunk "bursts" into a
    [128,1024] PSUM tile; the two MMs of a burst target distinct PE
    row-quadrants (tile_position) so they stream concurrently, and the
    4 GEMM matmuls interleaved between bursts keep the PE dense.
  - fmap multiply (psx * x0) is spread: DVE scalar_tensor_tensor
    straight from PSUM for 11 bursts, ScalarE-evacuate + GpSimd
    tensor_mul for 5 bursts per group.
  - GEMM: lhsT = W^T chunks [128c, 128o] (stationary), rhs = fmap chunk
    [128c, 512] (4 batches of K=128), accumulated over 32 chunks into
    2 single-buffered PSUM banks (O=256 -> 2 o-chunks), pipelined
    within the group with a 3-burst lag behind fmap construction.
  - Bias is added during PSUM evacuation via ScalarE activation.
"""

import sys
import types
import warnings

warnings.filterwarnings("ignore")

import numpy as np
import ml_dtypes

B, M, H, K, O = 256, 64, 64, 128, 256
C = H * M                  # 4096 channels
NCORES = 8
BPC = B // NCORES          # 32 batches per core
GRP = 4                    # batches per group (moving dim = GRP*K = 512)
NG = BPC // GRP            # 8 groups per core
KB = GRP * K               # 512
NCHUNK = C // 128          # 32 contraction chunks
NSUP = NCHUNK // 4         # 8 superchunks (chunk p reads xlp slice p//4)
NBURST = NCHUNK // 2       # 16 two-chunk bursts per group
GEMM_LAG = 3               # bursts of lag between fmap build and GEMM

_BF16 = ml_dtypes.bfloat16

LAST_EXEC_NS = None


def _install_ntff_hook():
    try:
        from antenv.axon_hooks import get_axon_ntff_profile_hook  # noqa: F401
        return
    except ImportError:
        pass
    try:
        from trn_agent_boot.trn_boot import _ntff_profile_via_ctypes
        hook = _ntff_profile_via_ctypes('/opt/axon/libaxon_pjrt.so')
    except Exception:
        hook = None
    m = types.ModuleType('antenv.axon_hooks')
    m.get_axon_ntff_profile_hook = lambda: hook
    m.set_axon_ntff_profile_hook = lambda h: None
    sys.modules['antenv.axon_hooks'] = m


_NC_CACHE = {}

# bursts handled by ScalarE-evac + GpSimd tensor_mul (rest: DVE STT)
_GP_BURSTS = frozenset((2, 5, 8, 11, 14))


def _build_program():
    if "nc" in _NC_CACHE:
        return _NC_CACHE["nc"]
    import concourse.bacc as bacc
    import concourse.tile as tile
    import concourse.mybir as mybir

    dt = mybir.dt
    nc = bacc.Bacc("TRN2", target_bir_lowering=False, debug=False)

    x0s_d = nc.dram_tensor("x0s", [NG, 128, 2 * KB], dt.bfloat16, kind="ExternalInput").ap()
    xlp_d = nc.dram_tensor("xlp", [NG, 8, NSUP * KB], dt.bfloat16, kind="ExternalInput").ap()
    wt_d = nc.dram_tensor("wt", [128, NCHUNK * O], dt.bfloat16, kind="ExternalInput").ap()
    e4_d = nc.dram_tensor("e4", [128, 128], dt.bfloat16, kind="ExternalInput").ap()
    bias_d = nc.dram_tensor("bias_t", [128, 2], dt.float32, kind="ExternalInput").ap()
    out_d = nc.dram_tensor("out", [BPC, O, K], dt.float32, kind="ExternalOutput").ap()

    with tile.TileContext(nc) as tc:
        with tc.tile_pool(name="const", bufs=1) as cpool, \
             tc.tile_pool(name="io", bufs=2) as iopool, \
             tc.tile_pool(name="fmapp", bufs=2) as fpool, \
             tc.tile_pool(name="xlbp", bufs=3) as xlbpool, \
             tc.tile_pool(name="outp", bufs=2) as opool, \
             tc.tile_pool(name="psx", bufs=3, space="PSUM") as psx, \
             tc.tile_pool(name="psg", bufs=1, space="PSUM") as psg:

            wt = cpool.tile([128, NCHUNK * O], dt.bfloat16)
            nc.sync.dma_start(wt[:], wt_d[:])
            e4 = cpool.tile([128, 128], dt.bfloat16)
            nc.sync.dma_start(e4[:], e4_d[:])
            # PE warmup: ~4.3us of dummy matmuls pulls the HAM clock-gate to
            # 8/8 before the first real broadcast/GEMM work lands.
            ps_w = psx.tile([128, 2 * KB], dt.float32, name="psx_warm", tag="psx")
            for wi in range(40):
                nc.tensor.matmul(ps_w[:, 0:128], e4[:, :], e4[:, :],
                                 start=(wi == 0), stop=(wi == 39))
            bias_t = cpool.tile([128, 2], dt.float32)
            nc.sync.dma_start(bias_t[:], bias_d[:])

            for g in range(NG):
                x0s = iopool.tile([128, 2 * KB], dt.bfloat16, name=f"x0s_{g}", tag="x0s")
                nc.sync.dma_start(x0s[:], x0s_d[g])
                xlp = iopool.tile([128, NSUP * KB], dt.bfloat16, name=f"xlp_{g}", tag="xlp")
                for i in range(4):
                    nc.sync.dma_start(xlp[32 * i:32 * i + 2, :], xlp_d[g, 2 * i:2 * i + 2, :])

                fmap = fpool.tile([128, NCHUNK * KB], dt.bfloat16, name=f"fmap_{g}", tag="fmap")

                pso = [psg.tile([128, KB], dt.float32, name=f"psg_{g}_{oc}", tag=f"psg{oc}")
                       for oc in range(2)]

                def emit_gemm_chunk(p):
                    for oc in range(2):
                        nc.tensor.matmul(pso[oc][:],
                                         wt[:, O * p + 128 * oc:O * p + 128 * (oc + 1)],
                                         fmap[:, KB * p:KB * (p + 1)],
                                         start=(p == 0), stop=(p == NCHUNK - 1))

                for b in range(NBURST):
                    # --- broadcast burst b: chunks 2b, 2b+1 into one psx tile
                    ps_x = psx.tile([128, 2 * KB], dt.float32, name=f"psx_{g}_{b}", tag="psx")
                    for j in range(2):
                        p = 2 * b + j
                        i = p % 4
                        s4 = p // 4
                        nc.tensor.matmul(ps_x[:, KB * j:KB * (j + 1)],
                                         e4[32 * i:32 * i + 2, :],
                                         xlp[32 * i:32 * i + 2, KB * s4:KB * (s4 + 1)],
                                         start=True, stop=True, tile_position=(32 * i, 0))
                    # --- consumer for burst b: fmap[2b:2b+2] = psx * x0
                    p0 = 2 * b
                    if b in _GP_BURSTS:
                        xlb = xlbpool.tile([128, 2 * KB], dt.bfloat16,
                                           name=f"xlb_{g}_{b}", tag="xlb")
                        nc.scalar.copy(xlb[:], ps_x[:])
                        nc.gpsimd.tensor_mul(fmap[:, KB * p0:KB * (p0 + 2)], xlb[:], x0s[:])
                    else:
                        nc.vector.scalar_tensor_tensor(
                            fmap[:, KB * p0:KB * (p0 + 2)], ps_x[:],
                            1.0, x0s[:], mybir.AluOpType.mult, mybir.AluOpType.mult)
                    # --- GEMM matmuls, lagged GEMM_LAG bursts behind
                    if b >= GEMM_LAG:
                        bb = b - GEMM_LAG
                        emit_gemm_chunk(2 * bb)
                        emit_gemm_chunk(2 * bb + 1)
                # GEMM tail: remaining chunks
                for bb in range(NBURST - GEMM_LAG, NBURST):
                    emit_gemm_chunk(2 * bb)
                    emit_gemm_chunk(2 * bb + 1)

                # evacuate GEMM PSUM with bias, DMA out
                for oc in range(2):
                    osb = opool.tile([128, KB], dt.float32, name=f"osb_{g}_{oc}", tag=f"osb{oc}")
                    nc.scalar.activation(osb[:], pso[oc][:],
                                         mybir.ActivationFunctionType.Identity,
                                         bias=bias_t[:, oc:oc + 1])
                    dst = out_d[GRP * g:GRP * (g + 1), 128 * oc:128 * (oc + 1), :] \
                        .rearrange("g o k -> o g k")
                    nc.sync.dma_start(dst, osb[:, :].rearrange("o (g k) -> o g k", k=K))

    nc.compile()
    _NC_CACHE["nc"] = nc
    return nc


def _host_prep(x0, xl, W, b):
    # x0s[core][g]: [128, 2*KB]  rows j = x0[b, j%64, :], cols (rep, gi*K+kk)
    # (b = 32c+4g+gi); duplicated along free so one op can span a 2-chunk burst.
    x0g = x0.reshape(NCORES, NG, GRP, M, K).transpose(0, 1, 3, 2, 4) \
        .reshape(NCORES, NG, M, KB)
    x0s = np.concatenate([x0g, x0g], axis=2)          # [NC, NG, 128, KB]
    x0s = np.concatenate([x0s, x0s], axis=3).astype(_BF16)  # [NC, NG, 128, 2KB]

    # xlp[core][g]: [8, NSUP*KB]; row 2i+r holds, at free offset s*KB + gi*K
    # + kk, the value xl[b(g,gi), 8s+2i+r, kk] (chunk p = 4s+i uses xl rows
    # {2p, 2p+1} = {8s+2i, 8s+2i+1}); DMA'd to SBUF partitions 32i+r.
    arr = xl.reshape(NCORES, NG, GRP, NSUP, 4, 2, K).transpose(0, 1, 4, 5, 3, 2, 6)
    xlp = np.ascontiguousarray(arr.reshape(NCORES, NG, 8, NSUP * KB)).astype(_BF16)

    Wm = W[:, :, 0]                        # [O, C]
    wt = np.ascontiguousarray(Wm.T).reshape(NCHUNK, 128, O).transpose(1, 0, 2) \
        .reshape(128, NCHUNK * O).astype(_BF16)   # wt[j, p*O+o] = W[o, 128p+j]

    e4 = np.zeros((128, 128), dtype=np.float32)
    for i in range(4):
        e4[32 * i + 0, 0:64] = 1.0
        e4[32 * i + 1, 64:128] = 1.0
    e4 = e4.astype(_BF16)

    bias_t = np.ascontiguousarray(b.reshape(2, 128).T.astype(np.float32))  # [128, 2]
    return x0s, xlp, wt, e4, bias_t


def kernel(x0, xl, k, W, b, _trace=False):
    global LAST_EXEC_NS
    _install_ntff_hook()
    import concourse.bass_utils as bass_utils

    x0 = np.asarray(x0, dtype=np.float32)
    xl = np.asarray(xl, dtype=np.float32)
    W = np.asarray(W, dtype=np.float32)
    b = np.asarray(b, dtype=np.float32)

    nc = _build_program()
    x0s, xlp, wt, e4, bias_t = _host_prep(x0, xl, W, b)

    in_maps = [
        {"x0s": np.ascontiguousarray(x0s[c]), "xlp": np.ascontiguousarray(xlp[c]),
         "wt": wt, "e4": e4, "bias_t": bias_t}
        for c in range(NCORES)
    ]
    res = bass_utils.run_bass_kernel_spmd(
        nc, in_maps, core_ids=list(range(NCORES)), trace=_trace)
    LAST_EXEC_NS = res.exec_time_ns

    out = np.concatenate([res.results[c]["out"][None] for c in range(NCORES)], axis=0)
    return np.ascontiguousarray(out.reshape(B, O, K)).astype(np.float32)


# revision 4
# speedup vs baseline: 1.3090x; 1.3090x over previous
"""Trainium2 Bass kernel for the CIN block:
out[b,o,k] = sum_{h,m} W[o, h*M+m] * xl[b,h,k] * x0[b,m,k] + bias[o]

Strategy (data-parallel over batch across 8 cores, 32 batches/core,
processed in 8 groups of 4 batches; GEMM operands bf16, fp32 PSUM).

The PE runs a warmup then ONE uninterrupted GEMM stream (8 groups x 64
matmuls, N=512) -- no broadcast matmuls, no PSUM traffic besides the
accumulators, which keeps the HAM clock-gate at 8/8 throughout.

fmap (the [C, K]-layout feature map chunks) is built one full group
ahead of the GEMM by DVE/GpSimd tensor_mul from two SBUF operands:
  - xlrep: xl rows pre-broadcast across the 64 m-partitions ON THE HOST
    and DMA'd in (4 MB/group, ~240 GB/s sustained -- DMA/AXI ports are
    physically separate from engine ports, so this is free time-wise).
  - x0s: x0 stacked twice along partitions, duplicated along free.
GEMM: lhsT = W^T chunks [128c, 128o], rhs = fmap chunk [128c, 512],
accumulated over 32 chunks into double-buffered PSUM banks (O=256 -> 2
o-chunks).  Bias is added during PSUM evacuation via ScalarE activation.
"""

import sys
import types
import warnings

warnings.filterwarnings("ignore")

import numpy as np
import ml_dtypes

B, M, H, K, O = 256, 64, 64, 128, 256
C = H * M                  # 4096 channels
NCORES = 8
BPC = B // NCORES          # 32 batches per core
GRP = 4                    # batches per group (moving dim = GRP*K = 512)
NG = BPC // GRP            # 8 groups per core
KB = GRP * K               # 512
NCHUNK = C // 128          # 32 contraction chunks
NBURST = NCHUNK // 2       # 16 two-chunk tensor_mul bursts per group
NPIECE = 4                 # xlrep DMA pieces per group (8 chunks each)

_BF16 = ml_dtypes.bfloat16

LAST_EXEC_NS = None


def _install_ntff_hook():
    try:
        from antenv.axon_hooks import get_axon_ntff_profile_hook  # noqa: F401
        return
    except ImportError:
        pass
    try:
        from trn_agent_boot.trn_boot import _ntff_profile_via_ctypes
        hook = _ntff_profile_via_ctypes('/opt/axon/libaxon_pjrt.so')
    except Exception:
        hook = None
    m = types.ModuleType('antenv.axon_hooks')
    m.get_axon_ntff_profile_hook = lambda: hook
    m.set_axon_ntff_profile_hook = lambda h: None
    sys.modules['antenv.axon_hooks'] = m


_NC_CACHE = {}

# bursts (of 2 chunks) handled by GpSimd tensor_mul (rest: DVE)
_GP_BURSTS = frozenset((2, 5, 8, 11, 14))


def _build_program():
    if "nc" in _NC_CACHE:
        return _NC_CACHE["nc"]
    import concourse.bacc as bacc
    import concourse.tile as tile
    import concourse.mybir as mybir

    dt = mybir.dt
    nc = bacc.Bacc("TRN2", target_bir_lowering=False, debug=False)

    x0s_d = nc.dram_tensor("x0s", [NG, 128, 2 * KB], dt.bfloat16, kind="ExternalInput").ap()
    xlr_d = nc.dram_tensor("xlr", [NG, NPIECE, 128, NCHUNK * KB // NPIECE],
                           dt.bfloat16, kind="ExternalInput").ap()
    wt_d = nc.dram_tensor("wt", [128, NCHUNK * O], dt.bfloat16, kind="ExternalInput").ap()
    wu_d = nc.dram_tensor("wu", [128, 128], dt.bfloat16, kind="ExternalInput").ap()
    bias_d = nc.dram_tensor("bias_t", [128, 2], dt.float32, kind="ExternalInput").ap()
    out_d = nc.dram_tensor("out", [BPC, O, K], dt.float32, kind="ExternalOutput").ap()

    PIECE = NCHUNK * KB // NPIECE      # 4096 cols = 8 chunks

    with tile.TileContext(nc) as tc:
        with tc.tile_pool(name="const", bufs=1) as cpool, \
             tc.tile_pool(name="io", bufs=2) as iopool, \
             tc.tile_pool(name="xlrp", bufs=2) as xlrpool, \
             tc.tile_pool(name="fmapp", bufs=2) as fpool, \
             tc.tile_pool(name="outp", bufs=2) as opool, \
             tc.tile_pool(name="psw", bufs=1, space="PSUM") as pswp, \
             tc.tile_pool(name="psg", bufs=2, space="PSUM") as psg:

            wu = cpool.tile([128, 128], dt.bfloat16)
            nc.sync.dma_start(wu[:], wu_d[:])
            bias_t = cpool.tile([128, 2], dt.float32)
            nc.sync.dma_start(bias_t[:], bias_d[:])

            x0s_t = [None] * NG
            xlr_t = [None] * NG           # per group: list of NPIECE tiles
            wt_t = [None] * NPIECE

            def dma_x0s(g):
                x0s_t[g] = iopool.tile([128, 2 * KB], dt.bfloat16,
                                       name=f"x0s_{g}", tag="x0s")
                nc.sync.dma_start(x0s_t[g][:], x0s_d[g])

            def dma_xlr_piece(g, q):
                if xlr_t[g] is None:
                    xlr_t[g] = [None] * NPIECE
                t = xlrpool.tile([128, PIECE], dt.bfloat16,
                                 name=f"xlr_{g}_{q}", tag=f"xlr{q}")
                nc.sync.dma_start(t[:], xlr_d[g, q])
                xlr_t[g][q] = t

            def dma_wt_piece(q):
                wt_t[q] = cpool.tile([128, NCHUNK * O // NPIECE], dt.bfloat16,
                                     name=f"wt_{q}", tag=f"wt{q}")
                sl = slice(q * NCHUNK * O // NPIECE, (q + 1) * NCHUNK * O // NPIECE)
                nc.sync.dma_start(wt_t[q][:], wt_d[:, sl])

            # startup DMA order: small stuff, then wt/xlrep interleaved so
            # arrival order matches consumption order.
            dma_x0s(0)
            dma_wt_piece(0)
            dma_xlr_piece(0, 0)
            dma_xlr_piece(0, 1)
            dma_wt_piece(1)
            dma_xlr_piece(0, 2)
            dma_wt_piece(2)
            dma_xlr_piece(0, 3)
            dma_wt_piece(3)

            # PE warmup: pulls the HAM clock-gate to 8/8 and covers the
            # initial input-DMA latency.
            ps_w = pswp.tile([128, KB], dt.float32, name="psx_warm", tag="psw")
            for wi in range(60):
                nc.tensor.matmul(ps_w[:, 0:128], wu[:, :], wu[:, :],
                                 start=(wi == 0), stop=(wi == 59))

            fmap_t = [None] * NG
            pso_t = [None] * NG

            def emit_fmap_build(g):
                # 16 two-chunk tensor_mul bursts: fmap = xlrep * x0s
                fmap_t[g] = fpool.tile([128, NCHUNK * KB], dt.bfloat16,
                                       name=f"fmap_{g}", tag="fmap")
                fmap = fmap_t[g]
                x0s = x0s_t[g]
                for b in range(NBURST):
                    q, r = divmod(b, NBURST // NPIECE)   # piece q, burst r in piece
                    src = xlr_t[g][q][:, 2 * KB * r:2 * KB * (r + 1)]
                    dst = fmap[:, 2 * KB * b:2 * KB * (b + 1)]
                    if b in _GP_BURSTS:
                        nc.gpsimd.tensor_mul(dst, src, x0s[:])
                    else:
                        nc.vector.tensor_mul(dst, src, x0s[:])

            def emit_gemm(g):
                pso_t[g] = [psg.tile([128, KB], dt.float32,
                                     name=f"psg_{g}_{oc}", tag=f"psg{oc}")
                            for oc in range(2)]
                pso = pso_t[g]
                fmap = fmap_t[g]
                for p in range(NCHUNK):
                    wtile = wt_t[p // (NCHUNK // NPIECE)]
                    wof = (p % (NCHUNK // NPIECE)) * O
                    for oc in range(2):
                        nc.tensor.matmul(pso[oc][:],
                                         wtile[:, wof + 128 * oc:wof + 128 * (oc + 1)],
                                         fmap[:, KB * p:KB * (p + 1)],
                                         start=(p == 0), stop=(p == NCHUNK - 1))
                for oc in range(2):
                    osb = opool.tile([128, KB], dt.float32,
                                     name=f"osb_{g}_{oc}", tag=f"osb{oc}")
                    nc.scalar.activation(osb[:], pso[oc][:],
                                         mybir.ActivationFunctionType.Identity,
                                         bias=bias_t[:, oc:oc + 1])
                    dst = out_d[GRP * g:GRP * (g + 1), 128 * oc:128 * (oc + 1), :] \
                        .rearrange("g o k -> o g k")
                    nc.sync.dma_start(dst, osb[:, :].rearrange("o (g k) -> o g k", k=K))

            emit_fmap_build(0)
            for g in range(NG):
                # prefetch inputs for g+1 (overwrites g-1's buffers)
                if g + 1 < NG:
                    dma_x0s(g + 1)
                    for q in range(NPIECE):
                        dma_xlr_piece(g + 1, q)
                    emit_fmap_build(g + 1)
                emit_gemm(g)

    nc.compile()
    _NC_CACHE["nc"] = nc
    return nc


def _host_prep(x0, xl, W, b):
    # x0s[core][g]: [128, 2*KB]  rows j = x0[b, j%64, :], cols (rep, gi*K+kk)
    # (b = 32c+4g+gi); duplicated along free so one op spans a 2-chunk burst.
    x0g = x0.reshape(NCORES, NG, GRP, M, K).transpose(0, 1, 3, 2, 4) \
        .reshape(NCORES, NG, M, KB)
    x0s = np.concatenate([x0g, x0g], axis=2)          # [NC, NG, 128, KB]
    x0s = np.concatenate([x0s, x0s], axis=3).astype(_BF16)  # [NC, NG, 128, 2KB]

    # xlrep[core][g]: [128, NCHUNK*KB]; partition q = (hh, m), free col
    # p*KB + gi*K + kk holds xl[b(g,gi), 2p+hh, kk] -- i.e. xl rows
    # broadcast across the 64 m partitions, host-side.
    xlb = xl.astype(_BF16)
    arr = xlb.reshape(NCORES, NG, GRP, NCHUNK, 2, K)       # [c,g,gi,p,hh,kk]
    arr = arr.transpose(0, 1, 4, 3, 2, 5)                  # [c,g,hh,p,gi,kk]
    # broadcast over m(64): target [c,g,hh,m,p,gi,kk]
    arr = np.broadcast_to(arr[:, :, :, None, :, :, :],
                          (NCORES, NG, 2, 64, NCHUNK, GRP, K))
    xlrep = np.ascontiguousarray(arr).reshape(NCORES, NG, 128, NCHUNK * KB)
    xlrep = xlrep.reshape(NCORES, NG, 128, NPIECE, NCHUNK * KB // NPIECE) \
        .transpose(0, 1, 3, 2, 4)          # [c, g, piece, 128, PIECE]
    xlrep = np.ascontiguousarray(xlrep)

    Wm = W[:, :, 0]                        # [O, C]
    wt = np.ascontiguousarray(Wm.T).reshape(NCHUNK, 128, O).transpose(1, 0, 2) \
        .reshape(128, NCHUNK * O).astype(_BF16)   # wt[j, p*O+o] = W[o, 128p+j]

    wu = np.zeros((128, 128), dtype=_BF16)
    bias_t = np.ascontiguousarray(b.reshape(2, 128).T.astype(np.float32))  # [128, 2]
    return x0s, xlrep, wt, wu, bias_t


def kernel(x0, xl, k, W, b, _trace=False):
    global LAST_EXEC_NS
    _install_ntff_hook()
    import concourse.bass_utils as bass_utils

    x0 = np.asarray(x0, dtype=np.float32)
    xl = np.asarray(xl, dtype=np.float32)
    W = np.asarray(W, dtype=np.float32)
    b = np.asarray(b, dtype=np.float32)

    nc = _build_program()
    x0s, xlrep, wt, wu, bias_t = _host_prep(x0, xl, W, b)

    in_maps = [
        {"x0s": np.ascontiguousarray(x0s[c]), "xlr": xlrep[c],
         "wt": wt, "wu": wu, "bias_t": bias_t}
        for c in range(NCORES)
    ]
    res = bass_utils.run_bass_kernel_spmd(
        nc, in_maps, core_ids=list(range(NCORES)), trace=_trace)
    LAST_EXEC_NS = res.exec_time_ns

    out = np.concatenate([res.results[c]["out"][None] for c in range(NCORES)], axis=0)
    return np.ascontiguousarray(out.reshape(B, O, K)).astype(np.float32)


# revision 5
# speedup vs baseline: 1.5632x; 1.1942x over previous
"""Trainium2 Bass kernel for the CIN block:
out[b,o,k] = sum_{h,m} W[o, h*M+m] * xl[b,h,k] * x0[b,m,k] + bias[o]

Strategy (data-parallel over batch across 8 cores, 32 batches/core,
processed in 8 groups of 4 batches; GEMM operands bf16, fp32 PSUM).

The PE runs a warmup then ONE uninterrupted GEMM stream (8 groups x 64
matmuls, N=512) -- no broadcast matmuls, no PSUM traffic besides the
accumulators, which keeps the HAM clock-gate at 8/8 throughout.

fmap (the [C, K]-layout feature map chunks) is built one full group
ahead of the GEMM by DVE/GpSimd tensor_mul from two SBUF operands:
  - xlrep: xl rows pre-broadcast across the 64 m-partitions ON THE HOST
    and DMA'd in (4 MB/group, ~240 GB/s sustained -- DMA/AXI ports are
    physically separate from engine ports, so this is free time-wise).
  - x0s: x0 stacked twice along partitions, duplicated along free.
GEMM: lhsT = W^T chunks [128c, 128o], rhs = fmap chunk [128c, 512],
accumulated over 32 chunks into double-buffered PSUM banks (O=256 -> 2
o-chunks).  Bias is added during PSUM evacuation via ScalarE activation.
"""

import sys
import types
import warnings

warnings.filterwarnings("ignore")

import numpy as np
import ml_dtypes

B, M, H, K, O = 256, 64, 64, 128, 256
C = H * M                  # 4096 channels
NCORES = 8
BPC = B // NCORES          # 32 batches per core
GRP = 4                    # batches per group (moving dim = GRP*K = 512)
NG = BPC // GRP            # 8 groups per core
KB = GRP * K               # 512
NCHUNK = C // 128          # 32 contraction chunks
NBURST = NCHUNK // 2       # 16 two-chunk tensor_mul bursts per group
NPIECE = 4                 # xlrep DMA pieces per group (8 chunks each)

_BF16 = ml_dtypes.bfloat16

LAST_EXEC_NS = None


def _install_ntff_hook():
    try:
        from antenv.axon_hooks import get_axon_ntff_profile_hook  # noqa: F401
        return
    except ImportError:
        pass
    try:
        from trn_agent_boot.trn_boot import _ntff_profile_via_ctypes
        hook = _ntff_profile_via_ctypes('/opt/axon/libaxon_pjrt.so')
    except Exception:
        hook = None
    m = types.ModuleType('antenv.axon_hooks')
    m.get_axon_ntff_profile_hook = lambda: hook
    m.set_axon_ntff_profile_hook = lambda h: None
    sys.modules['antenv.axon_hooks'] = m


_NC_CACHE = {}

# bursts (of 2 chunks) handled by GpSimd tensor_mul (rest: DVE)
_GP_BURSTS = frozenset()


def _build_program():
    if "nc" in _NC_CACHE:
        return _NC_CACHE["nc"]
    import concourse.bacc as bacc
    import concourse.tile as tile
    import concourse.mybir as mybir

    dt = mybir.dt
    nc = bacc.Bacc("TRN2", target_bir_lowering=False, debug=False)

    x0s_d = nc.dram_tensor("x0s", [NG, 128, 2 * KB], dt.bfloat16, kind="ExternalInput").ap()
    xlr_d = nc.dram_tensor("xlr", [NG, NPIECE, 128, NCHUNK * KB // NPIECE],
                           dt.bfloat16, kind="ExternalInput").ap()
    wt_d = nc.dram_tensor("wt", [128, NCHUNK * O], dt.bfloat16, kind="ExternalInput").ap()
    wu_d = nc.dram_tensor("wu", [128, 128], dt.bfloat16, kind="ExternalInput").ap()
    bias_d = nc.dram_tensor("bias_t", [128, 2], dt.float32, kind="ExternalInput").ap()
    out_d = nc.dram_tensor("out", [BPC, O, K], dt.float32, kind="ExternalOutput").ap()

    PIECE = NCHUNK * KB // NPIECE      # 4096 cols = 8 chunks

    with tile.TileContext(nc) as tc:
        with tc.tile_pool(name="const", bufs=1) as cpool, \
             tc.tile_pool(name="io", bufs=2) as iopool, \
             tc.tile_pool(name="xlrp", bufs=2) as xlrpool, \
             tc.tile_pool(name="fmapp", bufs=2) as fpool, \
             tc.tile_pool(name="outp", bufs=2) as opool, \
             tc.tile_pool(name="psw", bufs=1, space="PSUM") as pswp, \
             tc.tile_pool(name="psg", bufs=2, space="PSUM") as psg:

            wu = cpool.tile([128, 128], dt.bfloat16)
            nc.sync.dma_start(wu[:], wu_d[:])
            bias_t = cpool.tile([128, 2], dt.float32)
            nc.sync.dma_start(bias_t[:], bias_d[:])

            x0s_t = [None] * NG
            xlr_t = [None] * NG           # per group: list of NPIECE tiles
            wt_t = [None] * NPIECE

            def dma_x0s(g):
                x0s_t[g] = iopool.tile([128, 2 * KB], dt.bfloat16,
                                       name=f"x0s_{g}", tag="x0s")
                nc.sync.dma_start(x0s_t[g][:], x0s_d[g])

            def dma_xlr_piece(g, q):
                if xlr_t[g] is None:
                    xlr_t[g] = [None] * NPIECE
                t = xlrpool.tile([128, PIECE], dt.bfloat16,
                                 name=f"xlr_{g}_{q}", tag=f"xlr{q}")
                nc.sync.dma_start(t[:], xlr_d[g, q])
                xlr_t[g][q] = t

            def dma_wt_piece(q):
                wt_t[q] = cpool.tile([128, NCHUNK * O // NPIECE], dt.bfloat16,
                                     name=f"wt_{q}", tag=f"wt{q}")
                sl = slice(q * NCHUNK * O // NPIECE, (q + 1) * NCHUNK * O // NPIECE)
                nc.sync.dma_start(wt_t[q][:], wt_d[:, sl])

            # startup DMA order: small stuff, then wt/xlrep interleaved so
            # arrival order matches consumption order.
            dma_x0s(0)
            dma_wt_piece(0)
            dma_xlr_piece(0, 0)
            dma_xlr_piece(0, 1)
            dma_wt_piece(1)
            dma_xlr_piece(0, 2)
            dma_wt_piece(2)
            dma_xlr_piece(0, 3)
            dma_wt_piece(3)

            # PE warmup: pulls the HAM clock-gate to 8/8 and covers the
            # initial input-DMA latency.
            ps_w = pswp.tile([128, KB], dt.float32, name="psx_warm", tag="psw")
            for wi in range(60):
                nc.tensor.matmul(ps_w[:, 0:128], wu[:, :], wu[:, :],
                                 start=(wi == 0), stop=(wi == 59))

            fmap_t = [None] * NG
            pso_t = [None] * NG

            def emit_fmap_build(g):
                # 16 two-chunk tensor_mul bursts: fmap = xlrep * x0s
                fmap_t[g] = fpool.tile([128, NCHUNK * KB], dt.bfloat16,
                                       name=f"fmap_{g}", tag="fmap")
                fmap = fmap_t[g]
                x0s = x0s_t[g]
                for b in range(NBURST):
                    q, r = divmod(b, NBURST // NPIECE)   # piece q, burst r in piece
                    src = xlr_t[g][q][:, 2 * KB * r:2 * KB * (r + 1)]
                    dst = fmap[:, 2 * KB * b:2 * KB * (b + 1)]
                    if b in _GP_BURSTS:
                        nc.gpsimd.tensor_mul(dst, src, x0s[:])
                    else:
                        nc.vector.tensor_mul(dst, src, x0s[:])

            def emit_gemm(g):
                pso_t[g] = [psg.tile([128, KB], dt.float32,
                                     name=f"psg_{g}_{oc}", tag=f"psg{oc}")
                            for oc in range(2)]
                pso = pso_t[g]
                fmap = fmap_t[g]
                for p in range(NCHUNK):
                    wtile = wt_t[p // (NCHUNK // NPIECE)]
                    wof = (p % (NCHUNK // NPIECE)) * O
                    for oc in range(2):
                        nc.tensor.matmul(pso[oc][:],
                                         wtile[:, wof + 128 * oc:wof + 128 * (oc + 1)],
                                         fmap[:, KB * p:KB * (p + 1)],
                                         start=(p == 0), stop=(p == NCHUNK - 1))
                for oc in range(2):
                    osb = opool.tile([128, KB], dt.float32,
                                     name=f"osb_{g}_{oc}", tag=f"osb{oc}")
                    nc.scalar.activation(osb[:], pso[oc][:],
                                         mybir.ActivationFunctionType.Identity,
                                         bias=bias_t[:, oc:oc + 1])
                    dst = out_d[GRP * g:GRP * (g + 1), 128 * oc:128 * (oc + 1), :] \
                        .rearrange("g o k -> o g k")
                    nc.sync.dma_start(dst, osb[:, :].rearrange("o (g k) -> o g k", k=K))

            emit_fmap_build(0)
            for g in range(NG):
                # prefetch inputs for g+1 (overwrites g-1's buffers)
                if g + 1 < NG:
                    dma_x0s(g + 1)
                    for q in range(NPIECE):
                        dma_xlr_piece(g + 1, q)
                    emit_fmap_build(g + 1)
                emit_gemm(g)

    nc.compile()
    _NC_CACHE["nc"] = nc
    return nc


def _host_prep(x0, xl, W, b):
    # x0s[core][g]: [128, 2*KB]  rows j = x0[b, j%64, :], cols (rep, gi*K+kk)
    # (b = 32c+4g+gi); duplicated along free so one op spans a 2-chunk burst.
    x0g = x0.reshape(NCORES, NG, GRP, M, K).transpose(0, 1, 3, 2, 4) \
        .reshape(NCORES, NG, M, KB)
    x0s = np.concatenate([x0g, x0g], axis=2)          # [NC, NG, 128, KB]
    x0s = np.concatenate([x0s, x0s], axis=3).astype(_BF16)  # [NC, NG, 128, 2KB]

    # xlrep[core][g]: [128, NCHUNK*KB]; partition q = (hh, m), free col
    # p*KB + gi*K + kk holds xl[b(g,gi), 2p+hh, kk] -- i.e. xl rows
    # broadcast across the 64 m partitions, host-side.
    xlb = xl.astype(_BF16)
    arr = xlb.reshape(NCORES, NG, GRP, NCHUNK, 2, K)       # [c,g,gi,p,hh,kk]
    arr = arr.transpose(0, 1, 4, 3, 2, 5)                  # [c,g,hh,p,gi,kk]
    # broadcast over m(64): target [c,g,hh,m,p,gi,kk]
    arr = np.broadcast_to(arr[:, :, :, None, :, :, :],
                          (NCORES, NG, 2, 64, NCHUNK, GRP, K))
    xlrep = np.ascontiguousarray(arr).reshape(NCORES, NG, 128, NCHUNK * KB)
    xlrep = xlrep.reshape(NCORES, NG, 128, NPIECE, NCHUNK * KB // NPIECE) \
        .transpose(0, 1, 3, 2, 4)          # [c, g, piece, 128, PIECE]
    xlrep = np.ascontiguousarray(xlrep)

    Wm = W[:, :, 0]                        # [O, C]
    wt = np.ascontiguousarray(Wm.T).reshape(NCHUNK, 128, O).transpose(1, 0, 2) \
        .reshape(128, NCHUNK * O).astype(_BF16)   # wt[j, p*O+o] = W[o, 128p+j]

    wu = np.zeros((128, 128), dtype=_BF16)
    bias_t = np.ascontiguousarray(b.reshape(2, 128).T.astype(np.float32))  # [128, 2]
    return x0s, xlrep, wt, wu, bias_t


def kernel(x0, xl, k, W, b, _trace=False):
    global LAST_EXEC_NS
    _install_ntff_hook()
    import concourse.bass_utils as bass_utils

    x0 = np.asarray(x0, dtype=np.float32)
    xl = np.asarray(xl, dtype=np.float32)
    W = np.asarray(W, dtype=np.float32)
    b = np.asarray(b, dtype=np.float32)

    nc = _build_program()
    x0s, xlrep, wt, wu, bias_t = _host_prep(x0, xl, W, b)

    in_maps = [
        {"x0s": np.ascontiguousarray(x0s[c]), "xlr": xlrep[c],
         "wt": wt, "wu": wu, "bias_t": bias_t}
        for c in range(NCORES)
    ]
    res = bass_utils.run_bass_kernel_spmd(
        nc, in_maps, core_ids=list(range(NCORES)), trace=_trace)
    LAST_EXEC_NS = res.exec_time_ns

    out = np.concatenate([res.results[c]["out"][None] for c in range(NCORES)], axis=0)
    return np.ascontiguousarray(out.reshape(B, O, K)).astype(np.float32)
